# revision 2
# baseline (speedup 1.0000x reference)
"""BiRNN language-model kernel for 8 Trainium2 NeuronCores.

Strategy: data-parallel over the batch dim (B=32 -> 4 per core), no
collectives.  Per core:
  1. indirect-DMA gather of the core's S*4 embedding rows (natural order
     for the L->R scan, time-reversed order for the R->L scan)
  2. PE transposes -> x-projection matmul  xprojT[64, S*4]
     (rows 0:30 = W_e_lr^T emb, rows 32:62 = W_e_rl^T emb_rev; input
     biases ride the scan tanh as a per-partition bias AP)
  3. sequential scan, both directions stacked: one [64,64] block-diag
     matmul + identity matmul (xproj inject) + tanh per step
  4. output projection + log_softmax over V=32000 in two passes:
     pass1  logits -> PSUM, ACT exp with accum_out row sums -> logZ
     pass2  recompute logits, subtract logZ during the PSUM->SBUF copy,
     DMA 1MB tiles to HBM.

All SBUF access patterns start at partition 0/32/64/96 (hardware rule),
hence the direction blocks are padded from 30 to 32 partitions; pad rows
are zeroed and multiplied by zero weight rows so they never contribute.
"""

import sys

import numpy as np

for _p in ("/opt/trn_rl_repo", "/root/.axon_site/_ro/trn_rl_repo"):
    if _p not in sys.path:
        sys.path.insert(0, _p)

# problem constants
S, B, V, E, H = 128, 32, 32000, 150, 30
NCORES = 8
BL = B // NCORES          # batch rows per core
HP = 32                   # H padded to the 32-partition alignment
DH = 2 * HP               # 64: stacked direction state rows
KA = DH + 1               # 65: hcat rows + ones row (adds b_ho)
EH = 128                  # embedding dims handled by the "hi" K-split
EL = E - EH               # 22 remaining dims
VS = 512                  # fp32 matmul free-dim max (one PSUM bank)
SUP = 2048                # supertile: 4 PSUM banks, one ACT/DVE/DMA op


def _v_supertiles(v_total):
    tiles = []
    v0 = 0
    while v0 < v_total:
        w = min(SUP, v_total - v0)
        tiles.append((v0, w))
        v0 += w
    return tiles


def _splits512(w):
    out = []
    k0 = 0
    while k0 < w:
        kw = min(VS, w - k0)
        out.append((k0, kw))
        k0 += kw
    return out


def build_program(s=S, bl=BL, v=V):
    """Build the per-core Bass program (identical on all cores)."""
    from concourse import bacc, mybir
    import concourse.tile as tile

    f32 = mybir.dt.float32
    i32 = mybir.dt.int32
    Act = mybir.ActivationFunctionType

    r = s * bl                 # rows per core
    nch = r // 128             # 128-row chunks
    assert r % 128 == 0
    sup_tiles = _v_supertiles(v)
    ns = len(sup_tiles)

    nc = bacc.Bacc(None, target_bir_lowering=False)

    idx_lr_d = nc.dram_tensor("idx_lr", [128, nch], i32, kind="ExternalInput")
    idx_rl_d = nc.dram_tensor("idx_rl", [128, nch], i32, kind="ExternalInput")
    emb_d = nc.dram_tensor("emb", [V, E], f32, kind="ExternalInput")
    w_aug_d = nc.dram_tensor("w_aug", [KA, v], f32, kind="ExternalInput")
    we_lr_hi_d = nc.dram_tensor("we_lr_hi", [EH, H], f32, kind="ExternalInput")
    we_lr_lo_d = nc.dram_tensor("we_lr_lo", [EL, H], f32, kind="ExternalInput")
    we_rl_hi_d = nc.dram_tensor("we_rl_hi", [EH, H], f32, kind="ExternalInput")
    we_rl_lo_d = nc.dram_tensor("we_rl_lo", [EL, H], f32, kind="ExternalInput")
    wh_blk_d = nc.dram_tensor("wh_blk", [DH, DH], f32, kind="ExternalInput")
    i64_d = nc.dram_tensor("i64", [DH, DH], f32, kind="ExternalInput")
    ident_d = nc.dram_tensor("ident128", [128, 128], f32, kind="ExternalInput")
    init_d = nc.dram_tensor("init_stack", [DH, bl], f32, kind="ExternalInput")
    bias_d = nc.dram_tensor("bias_stack", [DH, 1], f32, kind="ExternalInput")
    out_d = nc.dram_tensor("out", [r, v], f32, kind="ExternalOutput")

    from concourse import bass

    with tile.TileContext(nc) as tc:
        with (
            tc.tile_pool(name="persist", bufs=1) as pp,
            tc.tile_pool(name="stage", bufs=3) as stp,
            tc.tile_pool(name="esc", bufs=2) as escp,
            tc.tile_pool(name="stat", bufs=2) as statp,
        ):
            # ---- input loads -------------------------------------------------
            w_aug_sb = pp.tile([KA, v], f32)
            nc.sync.dma_start(w_aug_sb[:], w_aug_d[:])
            ident = pp.tile([128, 128], f32)
            nc.sync.dma_start(ident[:], ident_d[:])
            idx_lr = pp.tile([128, nch], i32)
            nc.sync.dma_start(idx_lr[:], idx_lr_d[:])
            idx_rl = pp.tile([128, nch], i32)
            nc.sync.dma_start(idx_rl[:], idx_rl_d[:])
            we_lr_hi = pp.tile([EH, H], f32)
            nc.sync.dma_start(we_lr_hi[:], we_lr_hi_d[:])
            we_lr_lo = pp.tile([EL, H], f32)
            nc.sync.dma_start(we_lr_lo[:], we_lr_lo_d[:])
            we_rl_hi = pp.tile([EH, H], f32)
            nc.sync.dma_start(we_rl_hi[:], we_rl_hi_d[:])
            we_rl_lo = pp.tile([EL, H], f32)
            nc.sync.dma_start(we_rl_lo[:], we_rl_lo_d[:])
            wh_blk = pp.tile([DH, DH], f32)
            nc.sync.dma_start(wh_blk[:], wh_blk_d[:])
            i64 = pp.tile([DH, DH], f32)
            nc.sync.dma_start(i64[:], i64_d[:])
            init_sb = pp.tile([DH, bl], f32)
            nc.sync.dma_start(init_sb[:], init_d[:])
            bias_sb = pp.tile([DH, 1], f32)
            nc.sync.dma_start(bias_sb[:], bias_d[:])

            # ---- gather ------------------------------------------------------
            embg_lr = pp.tile([128, nch, E], f32)
            embg_rl = pp.tile([128, nch, E], f32)
            for j in range(nch):
                nc.gpsimd.indirect_dma_start(
                    out=embg_lr[:, j, :],
                    out_offset=None,
                    in_=emb_d[:],
                    in_offset=bass.IndirectOffsetOnAxis(ap=idx_lr[:, j : j + 1], axis=0),
                )
                nc.gpsimd.indirect_dma_start(
                    out=embg_rl[:, j, :],
                    out_offset=None,
                    in_=emb_d[:],
                    in_offset=bass.IndirectOffsetOnAxis(ap=idx_rl[:, j : j + 1], axis=0),
                )

            # ---- transposes + x-projection ----------------------------------
            embT_hi_lr = pp.tile([EH, r], f32)
            embT_hi_rl = pp.tile([EH, r], f32)
            embT_lo_lr = pp.tile([EL, r], f32)
            embT_lo_rl = pp.tile([EL, r], f32)

            xprojT = pp.tile([DH, r], f32)
            nc.vector.memset(xprojT[:], 0.0)
            hcat = pp.tile([KA, r], f32)
            nc.vector.memset(hcat[:], 0.0)
            nc.vector.memset(hcat[DH:KA, :], 1.0)

            with tc.tile_pool(name="pre_psum", bufs=2, space="PSUM") as prepsum:
                for embg, ehi, elo in (
                    (embg_lr, embT_hi_lr, embT_lo_lr),
                    (embg_rl, embT_hi_rl, embT_lo_rl),
                ):
                    for j in range(nch):
                        tp = prepsum.tile([128, 128], f32, tag="tp")
                        nc.tensor.transpose(tp[:], embg[:, j, 0:EH], ident[:])
                        nc.vector.tensor_copy(ehi[:, j * 128 : (j + 1) * 128], tp[:])
                        tp2 = prepsum.tile([128, 128], f32, tag="tp")
                        nc.tensor.transpose(tp2[0:EL, :], embg[:, j, EH:E], ident[:])
                        nc.vector.tensor_copy(elo[:, j * 128 : (j + 1) * 128], tp2[0:EL, :])

                for row0, whi, wlo, ehi, elo in (
                    (0, we_lr_hi, we_lr_lo, embT_hi_lr, embT_lo_lr),
                    (HP, we_rl_hi, we_rl_lo, embT_hi_rl, embT_lo_rl),
                ):
                    psx = prepsum.tile([H, r], f32, tag="xp")
                    nc.tensor.matmul(psx[:], whi[:], ehi[:], start=True, stop=False)
                    nc.tensor.matmul(psx[:], wlo[:], elo[:], start=False, stop=True)
                    nc.vector.tensor_copy(xprojT[row0 : row0 + H, :], psx[:])

                # ---- scan ---------------------------------------------------
                # hcat col layout: col t*bl+b
                #   rows 0:30  = hLR[t]  (pre-token-t state),  30:32 pad
                #   rows 32:62 = hRL[t+1],                     62:64 pad
                #   row  64    = ones (injects b_ho in the output matmul)
                nc.vector.tensor_copy(hcat[0:HP, 0:bl], init_sb[0:HP, :])
                nc.vector.tensor_copy(
                    hcat[HP:DH, (s - 1) * bl : s * bl], init_sb[HP:DH, :]
                )
                with tc.tile_pool(name="scanh", bufs=2) as shp:
                    hprev = init_sb
                    for t in range(s - 1):
                        ps = prepsum.tile([DH, bl], f32, tag="scan")
                        nc.tensor.matmul(ps[:], wh_blk[:], hprev[:], start=True, stop=False)
                        nc.tensor.matmul(
                            ps[:], i64[:], xprojT[:, t * bl : (t + 1) * bl],
                            start=False, stop=True,
                        )
                        hn = shp.tile([DH, bl], f32, tag="h")
                        nc.scalar.activation(hn[:], ps[:], Act.Tanh, bias=bias_sb[:, 0:1])
                        nc.vector.tensor_copy(
                            hcat[0:HP, (t + 1) * bl : (t + 2) * bl], hn[0:HP, :]
                        )
                        nc.vector.tensor_copy(
                            hcat[HP:DH, (s - 2 - t) * bl : (s - 1 - t) * bl],
                            hn[HP:DH, :],
                        )
                        hprev = hn

            # ---- output projection + log_softmax ----------------------------
            with tc.tile_pool(name="out_psum", bufs=2, space="PSUM") as opsum:
                for m in range(nch):
                    lhs = hcat[:, m * 128 : (m + 1) * 128]
                    sums = statp.tile([128, ns], f32, tag="sums")
                    for sti, (v0, w) in enumerate(sup_tiles):
                        ps = opsum.tile([128, SUP], f32, tag="ops")
                        for k0, kw in _splits512(w):
                            nc.tensor.matmul(
                                ps[:, k0 : k0 + kw], lhs,
                                w_aug_sb[:, v0 + k0 : v0 + k0 + kw],
                                start=True, stop=True,
                            )
                        esc = escp.tile([128, SUP], f32, tag="esc")
                        nc.scalar.activation(
                            esc[:, 0:w], ps[:, 0:w], Act.Exp,
                            accum_out=sums[:, sti : sti + 1],
                        )
                    sred = statp.tile([128, 1], f32, tag="sred")
                    nc.vector.tensor_reduce(
                        sred[:], sums[:, 0:ns],
                        axis=mybir.AxisListType.X, op=mybir.AluOpType.add,
                    )
                    lz = statp.tile([128, 1], f32, tag="lz")
                    nc.scalar.activation(lz[:], sred[:], Act.Ln)
                    for v0, w in sup_tiles:
                        ps = opsum.tile([128, SUP], f32, tag="ops")
                        for k0, kw in _splits512(w):
                            nc.tensor.matmul(
                                ps[:, k0 : k0 + kw], lhs,
                                w_aug_sb[:, v0 + k0 : v0 + k0 + kw],
                                start=True, stop=True,
                            )
                        stg = stp.tile([128, SUP], f32, tag="stg")
                        nc.vector.tensor_scalar_sub(stg[:, 0:w], ps[:, 0:w], lz[:, 0:1])
                        nc.sync.dma_start(
                            out_d[m * 128 : (m + 1) * 128, v0 : v0 + w], stg[:, 0:w]
                        )

    nc.compile()
    return nc


def prep_host_inputs(inputs, s=S, bl=BL, v=V, ncores=NCORES):
    """Slice/repack the full inputs into one in_map per core."""
    ib = np.asarray(inputs["input_batch"]).astype(np.int32)        # (s, B)
    emb = np.ascontiguousarray(np.asarray(inputs["embedding"], dtype=np.float32))
    W_lr = np.asarray(inputs["W_ih_lr"], dtype=np.float32)          # (E+H, H)
    b_lr = np.asarray(inputs["b_ih_lr"], dtype=np.float32)          # (1, H)
    W_rl = np.asarray(inputs["W_ih_rl"], dtype=np.float32)
    b_rl = np.asarray(inputs["b_ih_rl"], dtype=np.float32)
    W_ho = np.asarray(inputs["W_ho"], dtype=np.float32)             # (2H, v)
    b_ho = np.asarray(inputs["b_ho"], dtype=np.float32)             # (1, v)
    init = np.asarray(inputs["initial_hidden"], dtype=np.float32)   # (1, H)

    r = s * bl
    nch = r // 128

    w_aug = np.zeros((KA, v), np.float32)
    w_aug[0:H] = W_ho[0:H]
    w_aug[HP : HP + H] = W_ho[H : 2 * H]
    w_aug[DH] = b_ho[0]

    we_lr_hi = np.ascontiguousarray(W_lr[:EH])
    we_lr_lo = np.ascontiguousarray(W_lr[EH:E])
    we_rl_hi = np.ascontiguousarray(W_rl[:EH])
    we_rl_lo = np.ascontiguousarray(W_rl[EH:E])

    wh_blk = np.zeros((DH, DH), np.float32)
    wh_blk[:H, :H] = W_lr[E : E + H]
    wh_blk[HP : HP + H, HP : HP + H] = W_rl[E : E + H]
    i64 = np.eye(DH, dtype=np.float32)
    ident128 = np.eye(128, dtype=np.float32)

    init_stack = np.zeros((DH, bl), np.float32)
    init_stack[0:H] = init.T
    init_stack[HP : HP + H] = init.T
    bias_stack = np.zeros((DH, 1), np.float32)
    bias_stack[0:H, 0] = b_lr[0]
    bias_stack[HP : HP + H, 0] = b_rl[0]

    shared = {
        "emb": emb, "w_aug": w_aug,
        "we_lr_hi": we_lr_hi, "we_lr_lo": we_lr_lo,
        "we_rl_hi": we_rl_hi, "we_rl_lo": we_rl_lo,
        "wh_blk": wh_blk, "i64": i64, "ident128": ident128,
        "init_stack": init_stack, "bias_stack": bias_stack,
    }
    in_maps = []
    for c in range(ncores):
        ibc = ib[:, c * bl : (c + 1) * bl]                    # (s, bl)
        flat_lr = ibc.reshape(-1)                             # r = t*bl + b
        flat_rl = ibc[::-1].reshape(-1)
        idx_lr = np.ascontiguousarray(flat_lr.reshape(nch, 128).T)
        idx_rl = np.ascontiguousarray(flat_rl.reshape(nch, 128).T)
        in_maps.append(dict(shared, idx_lr=idx_lr, idx_rl=idx_rl))
    return in_maps


_CACHED = {}


def _get_program():
    if "nc" not in _CACHED:
        _CACHED["nc"] = build_program()
    return _CACHED["nc"]


def run_on_hw(inputs, trace=False):
    from concourse.bass_utils import run_bass_kernel_spmd

    nc = _get_program()
    in_maps = prep_host_inputs(inputs)
    res = run_bass_kernel_spmd(
        nc, in_maps, core_ids=list(range(NCORES)), trace=trace
    )
    out = np.empty((S, B, V), np.float32)
    for c in range(NCORES):
        out[:, c * BL : (c + 1) * BL, :] = res.results[c]["out"].reshape(S, BL, V)
    return out, res


def kernel(**inputs):
    out, _ = run_on_hw(inputs, trace=False)
    return out


# revision 3
# speedup vs baseline: 2.5332x; 2.5332x over previous
"""BiRNN language-model kernel for 8 Trainium2 NeuronCores.

Strategy: data-parallel over the batch dim (B=32 -> 4 per core), no
collectives.  Per core:
  1. indirect-DMA gather of the core's S*4 embedding rows (natural order
     for the L->R scan, time-reversed order for the R->L scan)
  2. PE transposes -> x-projection matmul  xprojT[64, S*4]
     (rows 0:30 = W_e_lr^T emb, rows 32:62 = W_e_rl^T emb_rev; input
     biases ride the scan tanh as a per-partition bias AP)
  3. sequential scan, both directions stacked.  The xproj sequence is
     pre-injected into one PSUM bank with a single identity matmul, so
     each step is ONE accumulating [64,64]@[64,4] matmul + tanh.
  4. output projection + log_softmax over V=32000 in two passes:
     pass1  logits -> PSUM, ACT exp with accum_out row sums -> logZ
     pass2  recompute logits, subtract logZ during the PSUM->SBUF copy,
     DMA 1MB tiles to HBM.

fp32 matmuls run 4 cycles/row (LOW_HIGH dual pass) on trn2, so all big
matmuls use fp16 operands with fp32 PSUM accumulation (measured end-to-end
rel err ~4e-5).  All SBUF access patterns start at partition 0/32/64/96
(hardware rule), hence the direction blocks are padded from 30 to 32
partitions; pad rows are zeroed weights so they never contribute.
"""

import sys

import numpy as np

for _p in ("/opt/trn_rl_repo", "/root/.axon_site/_ro/trn_rl_repo"):
    if _p not in sys.path:
        sys.path.insert(0, _p)

# problem constants
S, B, V, E, H = 128, 32, 32000, 150, 30
NCORES = 8
BL = B // NCORES          # batch rows per core
HP = 32                   # H padded to the 32-partition alignment
DH = 2 * HP               # 64: stacked direction state rows
KA = DH + 1               # 65: hcat rows + ones row (adds b_ho)
EH = 128                  # embedding dims handled by the "hi" K-split
EL = E - EH               # 22 remaining dims
VS = 512                  # fp32 matmul free-dim max (one PSUM bank)
SUP = 2048                # supertile: 4 PSUM banks, one ACT/DVE/DMA op


def _v_supertiles(v_total):
    tiles = []
    v0 = 0
    while v0 < v_total:
        w = min(SUP, v_total - v0)
        tiles.append((v0, w))
        v0 += w
    return tiles


def _splits512(w):
    out = []
    k0 = 0
    while k0 < w:
        kw = min(VS, w - k0)
        out.append((k0, kw))
        k0 += kw
    return out


# packed "smalls16" column layout (fp16, [128, n]):
#   we_lr_hi [128,30] | we_rl_hi [128,30] | we_lr_lo [22,30] | we_rl_lo [22,30]
#   | wh_blk [64,64] | i64 [64,64] | init [64,BL]
C_WLRH, C_WRLH, C_WLRL, C_WRLL = 0, 30, 60, 90
C_WHB, C_I64, C_INIT = 120, 184, 248
C_S16 = C_INIT + BL
# packed "smalls32" column layout (fp32, [128, n]): ident128 | bias [64,1]
C_IDENT, C_BIAS = 0, 128
C_S32 = 129


def build_program(s=S, bl=BL, v=V):
    """Build the per-core Bass program (identical on all cores)."""
    from concourse import bacc, mybir
    import concourse.tile as tile

    f32 = mybir.dt.float32
    f16 = mybir.dt.float16
    i32 = mybir.dt.int32
    Act = mybir.ActivationFunctionType

    r = s * bl                 # rows per core
    nch = r // 128             # 128-row chunks
    assert r % 128 == 0
    sup_tiles = _v_supertiles(v)
    ns = len(sup_tiles)
    c_init = C_INIT + bl

    nc = bacc.Bacc(None, target_bir_lowering=False)

    idx_d = nc.dram_tensor("idx", [128, 2 * nch], i32, kind="ExternalInput")
    emb_d = nc.dram_tensor("emb", [V, E], f32, kind="ExternalInput")
    w_aug_d = nc.dram_tensor("w_aug", [KA, v], f16, kind="ExternalInput")
    s16_d = nc.dram_tensor("smalls16", [128, c_init], f16, kind="ExternalInput")
    s32_d = nc.dram_tensor("smalls32", [128, C_S32], f32, kind="ExternalInput")
    out_d = nc.dram_tensor("out", [r, v], f32, kind="ExternalOutput")

    from concourse import bass

    with tile.TileContext(nc) as tc:
        with (
            tc.tile_pool(name="persist", bufs=1) as pp,
            tc.tile_pool(name="stage", bufs=3) as stp,
            tc.tile_pool(name="esc", bufs=2) as escp,
            tc.tile_pool(name="stat", bufs=2) as statp,
        ):
            # ---- input loads (idx first: the gather chain is the long pole)
            idx = pp.tile([128, 2 * nch], i32)
            nc.sync.dma_start(idx[:], idx_d[:])
            s16 = pp.tile([128, c_init], f16)
            nc.sync.dma_start(s16[:], s16_d[:])
            s32 = pp.tile([128, C_S32], f32)
            nc.sync.dma_start(s32[:], s32_d[:])
            w_aug_sb = pp.tile([KA, v], f16)
            nc.sync.dma_start(w_aug_sb[:], w_aug_d[:])

            ident = s32[:, C_IDENT : C_IDENT + 128]
            bias_ap = s32[0:DH, C_BIAS : C_BIAS + 1]
            we_lr_hi = s16[:, C_WLRH : C_WLRH + H]
            we_rl_hi = s16[:, C_WRLH : C_WRLH + H]
            we_lr_lo = s16[0:EL, C_WLRL : C_WLRL + H]
            we_rl_lo = s16[0:EL, C_WRLL : C_WRLL + H]
            wh_blk = s16[0:DH, C_WHB : C_WHB + DH]
            i64 = s16[0:DH, C_I64 : C_I64 + DH]
            init_sb = s16[0:DH, C_INIT : C_INIT + bl]

            # ---- gather ------------------------------------------------------
            embg_lr = pp.tile([128, nch, E], f32)
            embg_rl = pp.tile([128, nch, E], f32)
            for j in range(nch):
                nc.gpsimd.indirect_dma_start(
                    out=embg_lr[:, j, :],
                    out_offset=None,
                    in_=emb_d[:],
                    in_offset=bass.IndirectOffsetOnAxis(ap=idx[:, j : j + 1], axis=0),
                )
                nc.gpsimd.indirect_dma_start(
                    out=embg_rl[:, j, :],
                    out_offset=None,
                    in_=emb_d[:],
                    in_offset=bass.IndirectOffsetOnAxis(
                        ap=idx[:, nch + j : nch + j + 1], axis=0
                    ),
                )

            # ---- transposes + x-projection ----------------------------------
            embT_hi_lr = pp.tile([EH, r], f16)
            embT_hi_rl = pp.tile([EH, r], f16)
            embT_lo_lr = pp.tile([EL, r], f16)
            embT_lo_rl = pp.tile([EL, r], f16)

            xprojT = pp.tile([DH, r], f16)
            nc.vector.memset(xprojT[:], 0.0)
            hcat = pp.tile([KA, r], f16)
            nc.vector.memset(hcat[:], 0.0)
            nc.vector.memset(hcat[DH:KA, :], 1.0)

            with tc.tile_pool(name="pre_psum", bufs=2, space="PSUM") as prepsum:
                for embg, ehi, elo in (
                    (embg_lr, embT_hi_lr, embT_lo_lr),
                    (embg_rl, embT_hi_rl, embT_lo_rl),
                ):
                    for j in range(nch):
                        tp = prepsum.tile([128, 128], f32, tag="tp")
                        nc.tensor.transpose(tp[:], embg[:, j, 0:EH], ident)
                        nc.vector.tensor_copy(ehi[:, j * 128 : (j + 1) * 128], tp[:])
                        tp2 = prepsum.tile([128, 128], f32, tag="tp")
                        nc.tensor.transpose(tp2[0:EL, :], embg[:, j, EH:E], ident)
                        nc.vector.tensor_copy(elo[:, j * 128 : (j + 1) * 128], tp2[0:EL, :])

                for row0, whi, wlo, ehi, elo in (
                    (0, we_lr_hi, we_lr_lo, embT_hi_lr, embT_lo_lr),
                    (HP, we_rl_hi, we_rl_lo, embT_hi_rl, embT_lo_rl),
                ):
                    psx = prepsum.tile([H, r], f32, tag="xp")
                    nc.tensor.matmul(psx[:], whi, ehi[:], start=True, stop=False)
                    nc.tensor.matmul(psx[:], wlo, elo[:], start=False, stop=True)
                    nc.vector.tensor_copy(xprojT[row0 : row0 + H, :], psx[:])

            # ---- scan -------------------------------------------------------
            # hcat col layout: col t*bl+b
            #   rows 0:30  = hLR[t]  (pre-token-t state),  30:32 pad
            #   rows 32:62 = hRL[t+1],                     62:64 pad
            #   row  64    = ones (injects b_ho in the output matmul)
            nc.vector.tensor_copy(hcat[0:HP, 0:bl], init_sb[0:HP, :])
            nc.vector.tensor_copy(
                hcat[HP:DH, (s - 1) * bl : s * bl], init_sb[HP:DH, :]
            )
            with (
                tc.tile_pool(name="scan_psum", bufs=1, space="PSUM") as scp,
                tc.tile_pool(name="scanh", bufs=2) as shp,
            ):
                pscan = scp.tile([DH, VS], f32)
                nc.tensor.matmul(
                    pscan[:, 0 : (s - 1) * bl], i64, xprojT[:, 0 : (s - 1) * bl],
                    start=True, stop=False, skip_group_check=True,
                )
                hprev = init_sb
                for t in range(s - 1):
                    sl = slice(t * bl, (t + 1) * bl)
                    nc.tensor.matmul(
                        pscan[:, sl], wh_blk, hprev,
                        start=False, stop=(t == s - 2), skip_group_check=True,
                    )
                    hn = shp.tile([DH, bl], f16, tag="h")
                    nc.scalar.activation(hn[:], pscan[:, sl], Act.Tanh, bias=bias_ap)
                    nc.vector.tensor_copy(
                        hcat[0:HP, (t + 1) * bl : (t + 2) * bl], hn[0:HP, :]
                    )
                    nc.vector.tensor_copy(
                        hcat[HP:DH, (s - 2 - t) * bl : (s - 1 - t) * bl],
                        hn[HP:DH, :],
                    )
                    hprev = hn[:]

            # ---- output projection + log_softmax ----------------------------
            with tc.tile_pool(name="out_psum", bufs=2, space="PSUM") as opsum:
                for m in range(nch):
                    lhs = hcat[:, m * 128 : (m + 1) * 128]
                    sums = statp.tile([128, ns], f32, tag="sums")
                    for sti, (v0, w) in enumerate(sup_tiles):
                        ps = opsum.tile([128, SUP], f32, tag="ops")
                        for k0, kw in _splits512(w):
                            nc.tensor.matmul(
                                ps[:, k0 : k0 + kw], lhs,
                                w_aug_sb[:, v0 + k0 : v0 + k0 + kw],
                                start=True, stop=True,
                            )
                        esc = escp.tile([128, SUP], f32, tag="esc")
                        nc.scalar.activation(
                            esc[:, 0:w], ps[:, 0:w], Act.Exp,
                            accum_out=sums[:, sti : sti + 1],
                        )
                    sred = statp.tile([128, 1], f32, tag="sred")
                    nc.vector.tensor_reduce(
                        sred[:], sums[:, 0:ns],
                        axis=mybir.AxisListType.X, op=mybir.AluOpType.add,
                    )
                    lz = statp.tile([128, 1], f32, tag="lz")
                    nc.scalar.activation(lz[:], sred[:], Act.Ln)
                    for v0, w in sup_tiles:
                        ps = opsum.tile([128, SUP], f32, tag="ops")
                        for k0, kw in _splits512(w):
                            nc.tensor.matmul(
                                ps[:, k0 : k0 + kw], lhs,
                                w_aug_sb[:, v0 + k0 : v0 + k0 + kw],
                                start=True, stop=True,
                            )
                        stg = stp.tile([128, SUP], f32, tag="stg")
                        nc.vector.tensor_scalar_sub(stg[:, 0:w], ps[:, 0:w], lz[:, 0:1])
                        nc.sync.dma_start(
                            out_d[m * 128 : (m + 1) * 128, v0 : v0 + w], stg[:, 0:w]
                        )

    nc.compile()
    return nc


def prep_host_inputs(inputs, s=S, bl=BL, v=V, ncores=NCORES):
    """Slice/repack the full inputs into one in_map per core."""
    ib = np.asarray(inputs["input_batch"]).astype(np.int32)        # (s, B)
    emb = np.ascontiguousarray(np.asarray(inputs["embedding"], dtype=np.float32))
    W_lr = np.asarray(inputs["W_ih_lr"], dtype=np.float32)          # (E+H, H)
    b_lr = np.asarray(inputs["b_ih_lr"], dtype=np.float32)          # (1, H)
    W_rl = np.asarray(inputs["W_ih_rl"], dtype=np.float32)
    b_rl = np.asarray(inputs["b_ih_rl"], dtype=np.float32)
    W_ho = np.asarray(inputs["W_ho"], dtype=np.float32)             # (2H, v)
    b_ho = np.asarray(inputs["b_ho"], dtype=np.float32)             # (1, v)
    init = np.asarray(inputs["initial_hidden"], dtype=np.float32)   # (1, H)

    r = s * bl
    nch = r // 128
    c_init = C_INIT + bl

    w_aug = np.zeros((KA, v), np.float16)
    w_aug[0:H] = W_ho[0:H].astype(np.float16)
    w_aug[HP : HP + H] = W_ho[H : 2 * H].astype(np.float16)
    w_aug[DH] = b_ho[0].astype(np.float16)

    s16 = np.zeros((128, c_init), np.float16)
    s16[:, C_WLRH : C_WLRH + H] = W_lr[:EH]
    s16[:, C_WRLH : C_WRLH + H] = W_rl[:EH]
    s16[0:EL, C_WLRL : C_WLRL + H] = W_lr[EH:E]
    s16[0:EL, C_WRLL : C_WRLL + H] = W_rl[EH:E]
    s16[0:H, C_WHB : C_WHB + H] = W_lr[E : E + H]
    s16[HP : HP + H, C_WHB + HP : C_WHB + HP + H] = W_rl[E : E + H]
    s16[0:DH, C_I64 : C_I64 + DH] = np.eye(DH, dtype=np.float16)
    s16[0:H, C_INIT : c_init] = init.T
    s16[HP : HP + H, C_INIT : c_init] = init.T

    s32 = np.zeros((128, C_S32), np.float32)
    s32[:, C_IDENT : C_IDENT + 128] = np.eye(128, dtype=np.float32)
    s32[0:H, C_BIAS] = b_lr[0]
    s32[HP : HP + H, C_BIAS] = b_rl[0]

    shared = {"emb": emb, "w_aug": w_aug, "smalls16": s16, "smalls32": s32}
    in_maps = []
    for c in range(ncores):
        ibc = ib[:, c * bl : (c + 1) * bl]                    # (s, bl)
        flat_lr = ibc.reshape(-1)                             # r = t*bl + b
        flat_rl = ibc[::-1].reshape(-1)
        idxp = np.empty((128, 2 * nch), np.int32)
        idxp[:, 0:nch] = flat_lr.reshape(nch, 128).T
        idxp[:, nch : 2 * nch] = flat_rl.reshape(nch, 128).T
        in_maps.append(dict(shared, idx=idxp))
    return in_maps


_CACHED = {}


def _get_program():
    if "nc" not in _CACHED:
        _CACHED["nc"] = build_program()
    return _CACHED["nc"]


def run_on_hw(inputs, trace=False):
    from concourse.bass_utils import run_bass_kernel_spmd

    nc = _get_program()
    in_maps = prep_host_inputs(inputs)
    res = run_bass_kernel_spmd(
        nc, in_maps, core_ids=list(range(NCORES)), trace=trace
    )
    out = np.empty((S, B, V), np.float32)
    for c in range(NCORES):
        out[:, c * BL : (c + 1) * BL, :] = res.results[c]["out"].reshape(S, BL, V)
    return out, res


def kernel(**inputs):
    out, _ = run_on_hw(inputs, trace=False)
    return out
